# revision 25
# baseline (speedup 1.0000x reference)
"""Class-based decoder (MoE-style routing) on 8 trn2 NeuronCores.

Strategy: expert-parallel.  Classes are padded 250->256 and split 32 per
core.  On the host, tokens are grouped by class into capacity-padded slots
(C tokens per slot); overflow tokens beyond a class's capacity are evaluated
on the host (rare).  Each core receives:
  - xT   [128, n_mt*KCH*128]  its padded tokens, k-major, bf16, pre-scaled
         by 1/WSCALE
  - wcT  [128, KCH*256]       replicated class-decoder weights, k-major,
         bf16, pre-scaled by WSCALE
  - wwT  [n_mt, 128, per_mt*KCH*CHUNK]  its word-decoder shard, k-major,
         fp8 e3m4, pre-scaled by WSCALE
and computes class logits (x @ Wc.T) and word logits (x_c @ Ww[c].T) as PE
matmuls accumulating K=512 over 4 chunks.  The WSCALE=16 power-of-two
scaling moves the uniform(-0.1, 0.1) weights out of e3m4's subnormal range
(exact on x/wc since bf16 scaling by 2^k is lossless); the matmul
(W*16)@(x/16) needs no correction.

Word matmuls pair gs=2 adjacent class slots into one N=400 moving operand;
each 2C-row band's full 400-col pair block is stored ("wide") and the host
picks each row's own class half during the unpermute.  Output is bf16.

DMA plan: descriptor generation costs ~0.65us serialized per dma_start
per HWDGE queue, so the gens are split across both queues: x/wc (and the
output stores) on ACT, the W m-tiles on SP with the first m-tile halved.
The 16 DMA engines drain descriptors roughly FIFO across queues at
22.5 B/ns/engine, so the small x0/wc transfers complete while W0 streams.
Everything stays resident in SBUF (no pool recycling).  A run of gap-free
dummy warm-up matmuls burns through the PE clock governor's ramp
(0.65-1.2 GHz until several us of cumulative busy) before real data
arrives; any PE idle gap stalls the governor, so the schedule keeps the
PE stream contiguous from the first warm-up to the last matmul.
"""

import numpy as np
from contextlib import ExitStack

import concourse.bass as bass
import concourse.bacc as bacc
import concourse.tile as tile
import concourse.mybir as mybir
from concourse.bass_utils import run_bass_kernel_spmd

import ml_dtypes

NHID = 512
NCLS = 250
CHUNK = 200
NCORES = 8
KCH = NHID // 128          # 4 contraction chunks of 128
NCLS_PAD = 256             # classes padded so each core owns an equal shard
CPC = NCLS_PAD // NCORES   # classes per core
NCOL = NCLS + CHUNK        # 450 output columns
F32 = mybir.dt.float32
BF16 = mybir.dt.bfloat16
FP8 = mybir.dt.float8e3    # e3m4: 4 mantissa bits
NP_BF16 = ml_dtypes.bfloat16
NP_FP8 = ml_dtypes.float8_e3m4
WSCALE = 16.0              # 2^4: lifts |w|<=0.1 out of e3m4 subnormals

NWARM = 11                 # PE p-state warm-up matmuls

LAST_RESULT = None         # BassKernelResults of the most recent device run
_program_cache = {}


def _build_program(C, slots):
    n_mt = (slots * C) // 128  # 128-token m-tiles
    npad = slots * C
    per_mt = 128 // C          # class slots per m-tile
    gs = 2 if per_mt >= 2 else 1
    gw = gs * CHUNK            # word cols per pw matmul (400 paired)
    n_half = per_mt // gs      # pw matmuls per m-tile
    band = gs * C              # rows per pw matmul band (pair)
    ncls_p = 256
    ocol = NCLS + gw
    wchunk = KCH * gw          # free-dim elems per half inside a W tile

    nc = bacc.Bacc("TRN2", target_bir_lowering=False, debug=False,
                   num_devices=NCORES)
    xT = nc.dram_tensor("xT", [128, n_mt * KCH * 128], BF16,
                        kind="ExternalInput")
    wcT = nc.dram_tensor("wcT", [128, KCH * ncls_p], BF16,
                         kind="ExternalInput")
    wwT = nc.dram_tensor("wwT", [n_mt, 128, per_mt * KCH * CHUNK], FP8,
                         kind="ExternalInput")
    out = nc.dram_tensor("out", [npad, ocol], BF16, kind="ExternalOutput")

    with tile.TileContext(nc) as tc, ExitStack() as ctx:
        warmp = ctx.enter_context(tc.tile_pool(name="warm", bufs=1))
        xpool = ctx.enter_context(tc.tile_pool(name="x", bufs=1))
        wcpool = ctx.enter_context(tc.tile_pool(name="wc", bufs=1))
        wpool = ctx.enter_context(tc.tile_pool(name="w", bufs=n_mt))
        opool = ctx.enter_context(tc.tile_pool(name="o", bufs=n_mt))
        pcp = ctx.enter_context(
            tc.tile_pool(name="pc", bufs=2, space=bass.MemorySpace.PSUM))
        pwp = ctx.enter_context(
            tc.tile_pool(name="pw", bufs=6, space=bass.MemorySpace.PSUM))

        warm_sb = warmp.tile([128, 128 + gw], BF16)
        nc.gpsimd.memset(warm_sb[:], 0.0)

        # Loads: descriptor GENERATION is ~0.65us serial per dma_start per
        # queue, so split it: x/wc gens on the ACT queue (their descriptors
        # enter the shared engine FIFO first), W gens on the SP queue (W0
        # halved so its first half lands early, the rest whole).  The DMA
        # engines drain descriptors roughly FIFO, so the small x0/wc
        # transfers finish while W0 streams.  Stores also go on ACT (its
        # gens are done early).
        x_sb = xpool.tile([128, n_mt * KCH * 128], BF16)
        wc_sb = wcpool.tile([128, KCH * ncls_p], BF16)
        w_sbs = [wpool.tile([128, per_mt * KCH * CHUNK], FP8, tag="w",
                            name=f"w_sb{m}")
                 for m in range(n_mt)]
        nc.scalar.dma_start(x_sb[:, :KCH * 128], xT[:, :KCH * 128])
        nc.scalar.dma_start(wc_sb[:], wcT[:])
        for m in range(1, n_mt):
            nc.scalar.dma_start(
                x_sb[:, m * KCH * 128:(m + 1) * KCH * 128],
                xT[:, m * KCH * 128:(m + 1) * KCH * 128])
        wlen = per_mt * KCH * CHUNK
        for m in range(n_mt):
            nsplit = 2 if (m == 0 and n_half % 2 == 0) else 1
            step = wlen // nsplit
            for s in range(nsplit):
                nc.sync.dma_start(
                    w_sbs[m][:, s * step:(s + 1) * step],
                    wwT[m][:, s * step:(s + 1) * step])

        # PE p-state warm-up: garbage matmuls with no DMA dependencies.
        # They must run BACK TO BACK (pw pool rotation keeps them gap-free;
        # any semaphore gap stalls the clock governor at its lowest step).
        # The 2.4GHz boost only engages after ~9us of cumulative gap-free
        # PE busy, so every idle-filling cycle before the loads land counts.
        for i in range(NWARM):
            wps = pwp.tile([128, gw], F32, tag="pw", name=f"warm{i}")
            nc.tensor.matmul(wps[:, :], warm_sb[:, :128],
                             warm_sb[:, 128:128 + gw], start=True, stop=True)

        for m in range(n_mt):
            def xcol(j):
                base = (m * KCH + j) * 128
                return x_sb[:, base:base + 128]

            o_sb = opool.tile([128, ocol], BF16)

            def do_class():
                pc_ps = pcp.tile([128, ncls_p], F32, tag="pc")
                for j in range(KCH):
                    nc.tensor.matmul(
                        pc_ps[:, :],
                        xcol(j),
                        wc_sb[:, j * ncls_p:(j + 1) * ncls_p],
                        start=(j == 0), stop=(j == KCH - 1),
                    )
                nc.vector.tensor_copy(o_sb[:, :NCLS], pc_ps[:, :NCLS])

            def do_words():
                # on the last m-tile, odd bands copy on DVE so the ACT
                # queue is free to generate the first store while the
                # final band's copy drains
                swap = (m == n_mt - 1)
                for h in range(n_half):
                    pw_ps = pwp.tile([128, gw], F32, tag="pw")
                    for j in range(KCH):
                        nc.tensor.matmul(
                            pw_ps[:, :],
                            xcol(j),
                            w_sbs[m][:, h * wchunk + j * gw:
                                     h * wchunk + (j + 1) * gw],
                            start=(j == 0), stop=(j == KCH - 1),
                        )
                    if (h % 2 == 0) != swap:
                        nc.vector.tensor_copy(
                            o_sb[h * band:(h + 1) * band, NCLS:],
                            pw_ps[h * band:(h + 1) * band, :])
                    else:
                        nc.scalar.copy(o_sb[h * band:(h + 1) * band, NCLS:],
                                       pw_ps[h * band:(h + 1) * band, :])

            # m0's words can start before wc lands; for later m-tiles the
            # class matmul inputs are there long before the W tile, so run
            # class first (keeps its copy off the store's critical path).
            if m == 0:
                do_words()
                do_class()
            else:
                do_class()
                do_words()

            if m == n_mt - 1 and n_half >= 2:
                # split the final store so the bulk of the last m-tile
                # ships while the last band's copy is still finishing
                rsplit = (n_half - 1) * band
                nc.scalar.dma_start(out[m * 128:m * 128 + rsplit, :],
                                    o_sb[:rsplit, :])
                nc.scalar.dma_start(out[m * 128 + rsplit:(m + 1) * 128, :],
                                    o_sb[rsplit:, :])
            else:
                nc.scalar.dma_start(out[m * 128:(m + 1) * 128, :], o_sb[:])

    nc.compile()
    return nc


def _route(cls):
    """Group tokens by class into capacity-padded slots: one slot per class,
    C tokens of capacity.  Tokens beyond a class's capacity are returned as
    `overflow` and evaluated on the host in numpy.

    Returns (C, slots, tok_idx [NCORES, slots*C] int64 token id or -1,
    slot_cls [NCORES, slots] class id per slot, overflow token-id array).
    """
    counts = np.bincount(cls, minlength=NCLS_PAD)
    cands = (16, 32, 64, 128)
    C = cands[-1]
    for c in cands:
        if int(np.maximum(counts - c, 0).sum()) <= 32:
            C = c
            break

    order = np.argsort(cls, kind="stable")
    starts = np.zeros(NCLS_PAD + 1, np.int64)
    starts[1:] = np.cumsum(counts)

    slots = CPC  # one slot per class owned by the core
    tok_idx = np.full((NCORES, slots * C), -1, np.int64)
    slot_cls = np.full((NCORES, slots), -1, np.int64)
    overflow = []
    for k in range(NCORES):
        for s in range(slots):
            c = k * CPC + s
            lo, cnt = int(starts[c]), int(counts[c])
            n = min(C, cnt)
            slot_cls[k, s] = c
            if n > 0:
                tok_idx[k, s * C:s * C + n] = order[lo:lo + n]
            if cnt > C:
                overflow.append(order[lo + C:lo + cnt])
    overflow = (np.concatenate(overflow) if overflow
                else np.zeros((0,), np.int64))
    return C, slots, tok_idx, slot_cls, overflow


def kernel(x, Wc, bc, Ww, bw, cls_idx, _trace=False, _trace_cores=None):
    global LAST_RESULT

    x = np.ascontiguousarray(np.asarray(x, np.float32))
    Wc = np.ascontiguousarray(np.asarray(Wc, np.float32))
    bc = np.asarray(bc, np.float32)
    Ww = np.ascontiguousarray(np.asarray(Ww, np.float32))
    bw = np.asarray(bw, np.float32)
    cls = np.asarray(cls_idx).astype(np.int64).ravel()
    N = cls.shape[0]

    C, slots, tok_idx, slot_cls, overflow = _route(cls)
    npad = slots * C
    n_mt = npad // 128
    per_mt = 128 // C
    gs = 2 if per_mt >= 2 else 1
    gw = gs * CHUNK
    ncls_p = 256
    ocol = NCLS + gw

    key = (C, slots)
    if key not in _program_cache:
        _program_cache[key] = _build_program(C, slots)
    nc = _program_cache[key]

    # wcT [128, KCH*256]: wcT[p, j*256+c] = Wc[c, j*128+p] * WSCALE
    Wc_p = np.concatenate(
        [Wc, np.zeros((ncls_p - NCLS, NHID), np.float32)], 0) * WSCALE
    wcT = np.ascontiguousarray(
        Wc_p.reshape(ncls_p, KCH, 128).transpose(2, 1, 0)
            .reshape(128, KCH * ncls_p).astype(NP_BF16))

    Ww_pad = np.zeros((NCLS_PAD, CHUNK, NHID), np.float32)
    Ww_pad[:NCLS] = Ww * WSCALE

    in_maps = []
    for k in range(NCORES):
        # per-slot k-major weights: tmp[s, j, p, w] = Ww[cls_s, w, j*128+p]
        wsel = Ww_pad[np.maximum(slot_cls[k], 0)]
        wsel[slot_cls[k] < 0] = 0.0
        tmp = wsel.reshape(slots, CHUNK, KCH, 128).transpose(0, 2, 3, 1)
        if gs == 2:
            # group = m-tile; within: pair h, then j, then the two slots'
            # CHUNK columns side by side
            tmp = tmp.reshape(n_mt, per_mt // 2, 2, KCH, 128, CHUNK)
            tmp = tmp.transpose(0, 4, 1, 3, 2, 5)  # [n_mt,128,pair,j,2,CHUNK]
        else:
            tmp = tmp.reshape(n_mt, per_mt, KCH, 128, CHUNK)
            tmp = tmp.transpose(0, 3, 1, 2, 4)     # [n_mt,128,q,j,CHUNK]
        wwT = np.ascontiguousarray(
            tmp.reshape(n_mt, 128, per_mt * KCH * CHUNK).astype(NP_FP8))

        ti = tok_idx[k]
        xk = x[np.maximum(ti, 0)] * (1.0 / WSCALE)
        xk[ti < 0] = 0.0
        # xT[p, (m*KCH+j)*128 + t] = xk[m*128+t, j*128+p]
        xT = np.ascontiguousarray(
            xk.reshape(n_mt, 128, KCH, 128).transpose(3, 0, 2, 1)
              .reshape(128, n_mt * KCH * 128).astype(NP_BF16))
        in_maps.append({"xT": xT, "wcT": wcT, "wwT": wwT})

    LAST_RESULT = run_bass_kernel_spmd(
        nc, in_maps, list(range(NCORES)), trace=_trace,
        trace_cores=(_trace_cores if _trace else None))

    out = np.zeros((N, NCOL), np.float32)
    if gs == 2:
        # row r's slot parity selects which CHUNK half is its class
        a_row = (np.arange(npad) // C) % 2
    for k in range(NCORES):
        ok = np.asarray(LAST_RESULT.results[k]["out"], np.float32)
        if gs == 2:
            words = np.where((a_row == 0)[:, None],
                             ok[:, NCLS:NCLS + CHUNK],
                             ok[:, NCLS + CHUNK:NCLS + 2 * CHUNK])
            ok = np.concatenate([ok[:, :NCLS], words], 1)
        valid = tok_idx[k] >= 0
        out[tok_idx[k][valid]] = ok[valid]

    if overflow.size:
        # rare capacity-overflow tokens: evaluate directly on the host
        xo = x[overflow]                                   # [no, NHID]
        out[overflow, :NCLS] = xo @ Wc.T
        co = cls[overflow]
        out[overflow, NCLS:] = np.einsum(
            "nkh,nh->nk", Ww[co], xo, optimize=True)

    out[:, :NCLS] += bc
    out[:, NCLS:] += bw[cls]
    return out
